# revision 37
# baseline (speedup 1.0000x reference)
"""Fused multi-head attention block (QKV proj + softmax attention + out proj
+ LN + relu-residual + LN) for Trainium2, SPMD across 8 NeuronCores.

Problem shapes (hardcoded): B=2, NQ=NK=4096, D=256, H=8, DH=32.

Sharding: sequence-parallel over (batch, query-chunk): core c handles batch
c//4, query rows [1024*(c%4), 1024*(c%4+1)). Each core reads the K rows of
its batch and computes its query chunk end-to-end. No collectives.

Engine plan (HW-calibrated: tile-packed matmuls at distinct tile_positions
execute CONCURRENTLY on the PE quads, so the packed scores/attn/den matmuls
cost ~1/4 of their serial instruction time; output-partition-narrow matmuls
are fine when packed; weight reloads for full [128,128] stationary tiles
are NOT free, so the attn@V stays in the V-stationary orientation):
  PE    : projections; scoresT (4x row-packed K=32); attn@V + denominator
          (4x col-packed each, [32,512] out, V/ones stationary); out-proj.
  ACT   : exp of score tile A (native Exp, one [128,1024] instr/j) and of
          tile B every ACTB-th j; phase-A PSUM->SBUF copies; a = y - mu
          (bias-add); one batched Ln+Exp rsqrt for all 8 tail blocks per
          iteration (kills the per-tail act-table thrash, ~17 table loads).
  DVE   : exp of score tile B via 1-pass custom cubic, bn_stats/aggr,
          softmax reciprocal + normalize.
  Pool  : SBUF-only tail elementwise: -mu, relu/add, d calc, final scale.

Pipelining: av/dn run AVLAG j's behind the score/exp stream (u-tile ring
depth 2*(AVLAG+1)) so PE never stalls on a single exp's jitter. kpt/vp are
split into per-chunk tiles so phase-B reads only wait on the chunk they
touch (finer deps let phase A overlap the previous iteration's tail). The
tail keeps only PSUM-touching work inline per q-block; the SBUF-only
rsqrt+scale+store runs once at iteration end, so the next iteration's
phase A isn't gated behind it by PSUM pool aliasing.

Fused tail: with t = (y-mu)*rs (LN0) and z = t + relu(t) (= rs*w where
w = max(2a, a), a = y-mu), LN1(z) = (w - mean(w)) * rsqrt(var(w) +
eps*(var(y)+eps)) -- LN0's rs never needs to be computed.

Knobs (env): ACTB=0 (ACT additionally takes score tile B every ACTB-th j),
ACOPY=act (phase-A copy engine), AVLAG=2, UB (u ring, default from AVLAG).
"""

import os

import numpy as np

import concourse.bass as bass
import concourse.mybir as mybir
import concourse.tile as tile
from concourse import bacc
from concourse.bass_utils import run_bass_kernel_spmd

F32 = mybir.dt.float32
F32R = mybir.dt.float32r
BF16 = mybir.dt.bfloat16
AF = mybir.ActivationFunctionType
ALU = mybir.AluOpType

B, NQ, NK = 2, 4096, 4096
D = 256
H = 8
DH = 32
LN_EPS = 1e-5
NCORES = 8
QC = (B * NQ) // NCORES  # 1024 query rows per core
SCALE = 1.0 / np.sqrt(np.float32(DH))
NJ = NK // 128  # 32 k-tiles

_DVE_OPS = {}


def _register_dve_ops():
    """Runtime-register the custom DVE ops used by this kernel."""
    if _DVE_OPS:
        return _DVE_OPS
    import concourse.dve_ops as dve_ops
    from concourse.dve_spec import (
        C0, C1, C2, C3, Spec, Src0, _spill_c3_to_src1, lower,
    )
    from concourse.dve_uop import DveOpSpec

    def _mk(name, spec, rd1_en):
        for op in dve_ops.OPS:
            if op.name == name:
                return op
        row = dve_ops._CUSTOM_DVE_ROW_BASE + len(dve_ops.OPS)
        shas = {}
        for ver in ("v3", "v4"):
            tmp = DveOpSpec(name=name, opcode=row, uops=lower(spec, ver=ver),
                            rd1_en=rd1_en)
            shas[ver] = tmp.sha(ver)
        op = dve_ops.DveOp(name, spec, subdim=False, uops_sha=shas)
        dve_ops.OPS.append(op)
        dve_ops.CUSTOM_DVE_SPECS[op.name] = op.spec
        dve_ops._SUB_OPCODE_FOR_NAME[op.name] = row
        return op

    # cubic exp: out = ((c3*x + c2)*x + c1)*x + c0, c3 rides in1 ([P,1])
    def _exp3_ref(in0, in1, c0, c1, c2):
        c3 = in1[:, :1]
        x = in0.astype(np.float32)
        return ((c3 * x + c2) * x + c1) * x + c0

    exp3 = _mk(
        "EXP3_ANT",
        Spec(
            body=_spill_c3_to_src1(((C3 * Src0 + C2) * Src0 + C1) * Src0 + C0),
            reference=_exp3_ref,
        ),
        rd1_en=True,
    )
    _DVE_OPS["exp3"] = exp3
    return _DVE_OPS


def _fit_exp_cubic(scale, hi_raw):
    """Chebyshev-node cubic fit of e^(scale*x) for x in [-hi_raw, hi_raw]
    (raw, unscaled scores). Returns (c0, c1, c2, c3)."""
    t = np.cos(np.linspace(0, np.pi, 20001))
    xc = hi_raw * t
    yc = np.exp(np.float64(scale) * xc)
    c = np.polyfit(xc, yc, 3)
    return tuple(float(v) for v in c[::-1])


def _build_kernel(trivial_affine, repeat=1):
    """Build the SPMD Bass program. trivial_affine: all biases zero, all LN
    gammas one, betas zero (true for this problem's setup_inputs)."""
    ops = _register_dve_ops()
    exp3 = ops["exp3"]
    c0, c1, c2, c3 = _fit_exp_cubic(SCALE, 4.6)
    # ACT additionally takes score tile B every ACTB-th j (0 = never)
    ACTB = int(os.environ.get("ACTB", "0"))
    # phase-A PSUM->SBUF copy engine: act | dve
    ACOPY = os.environ.get("ACOPY", "act")
    # DVE additionally takes score tile A every DVEB-th j (0 = never)
    DVEB = int(os.environ.get("DVEB", "0"))
    AVLAG = int(os.environ.get("AVLAG", "2"))
    UB = int(os.environ.get("UB", str(2 * (int(os.environ.get("AVLAG", "2")) + 1))))

    nc = bacc.Bacc("TRN2", target_bir_lowering=False)

    # ---- dram i/o ----
    qT = nc.dram_tensor("qT", [D, QC], F32R, kind="ExternalInput")
    kT = nc.dram_tensor("kT", [D, NK], F32R, kind="ExternalInput")
    wqT = nc.dram_tensor("wqT", [D, D], F32R, kind="ExternalInput")
    wkT = nc.dram_tensor("wkT", [D, D], F32R, kind="ExternalInput")
    wvT = nc.dram_tensor("wvT", [D, D], F32R, kind="ExternalInput")
    woT = nc.dram_tensor("woT", [D, D], F32R, kind="ExternalInput")
    # vecsP[d, i]: per-partition-use vectors; col 0=bq, 1=bv
    vecsP = nc.dram_tensor("vecsP", [D, 2], F32, kind="ExternalInput")
    # vecsF[i, d]: free-dim-use vectors; row 0=bo 1=g0 2=beta0 3=g1 4=beta1
    vecsF = nc.dram_tensor("vecsF", [5, D], F32, kind="ExternalInput")
    out = nc.dram_tensor("out", [QC, D], F32, kind="ExternalOutput")

    with tile.TileContext(nc) as tc:
        with tc.tile_pool(name="sb", bufs=1) as sb:
            # ---- load inputs (q/k chunked so phase A streams behind) ----
            qt = [sb.tile([128, QC], F32R, tag=f"qt{i}", name=f"qt{i}") for i in range(2)]
            kt = [sb.tile([128, NK], F32R, tag=f"kt{i}", name=f"kt{i}") for i in range(2)]
            wqt = [sb.tile([128, D], F32R, tag=f"wqt{i}", name=f"wqt{i}") for i in range(2)]
            wkt = [sb.tile([128, D], F32R, tag=f"wkt{i}", name=f"wkt{i}") for i in range(2)]
            wvt = [sb.tile([128, D], F32R, tag=f"wvt{i}", name=f"wvt{i}") for i in range(2)]
            wot = [sb.tile([128, D], F32R, tag=f"wot{i}", name=f"wot{i}") for i in range(2)]
            ones32 = sb.tile([128, 32], BF16)
            c3t = sb.tile([128, 1], F32)
            epst = sb.tile([128, 1], F32)
            vp_ = [sb.tile([128, 2], F32, tag=f"vp_{i}", name=f"vp_{i}") for i in range(2)]
            vf_ = sb.tile([128, 5, D], F32) if not trivial_affine else None
            # per-tail-block persistent state (written each iter, read at end)
            wws = sb.tile([128, 8, D], F32)
            mvbs = sb.tile([128, 8, 2], F32)
            dds = sb.tile([128, 8], F32)
            for i in range(2):
                nc.sync.dma_start(out=wqt[i], in_=wqT[128 * i : 128 * i + 128, :])
                nc.sync.dma_start(out=wkt[i], in_=wkT[128 * i : 128 * i + 128, :])
                nc.sync.dma_start(out=wvt[i], in_=wvT[128 * i : 128 * i + 128, :])
                nc.sync.dma_start(out=wot[i], in_=woT[128 * i : 128 * i + 128, :])
                nc.sync.dma_start(out=qt[i], in_=qT[128 * i : 128 * i + 128, :])
                for ck in range(4):
                    nc.sync.dma_start(
                        out=kt[i][:, 1024 * ck : 1024 * ck + 1024],
                        in_=kT[128 * i : 128 * i + 128,
                              1024 * ck : 1024 * ck + 1024],
                    )
                nc.sync.dma_start(out=vp_[i], in_=vecsP[128 * i : 128 * i + 128, :])
            nc.vector.memset(ones32, 1.0)
            if vf_ is not None:
                nc.gpsimd.dma_start(
                    out=vf_, in_=vecsF[:, :].unsqueeze(0).broadcast_to([128, 5, D])
                )
            nc.vector.memset(c3t, c3)
            nc.vector.memset(epst, LN_EPS)
            # warm the ACT exp/ln table set while input DMAs stream
            warmt = sb.tile([128, 1], F32)
            nc.scalar.activation(out=warmt, in_=epst, func=AF.Exp)
            nc.scalar.activation(out=warmt, in_=warmt, func=AF.Ln)

            qpt = [sb.tile([128, QC], BF16, tag=f"qpt{g}", name=f"qpt{g}")
                   for g in range(2)]
            # kpt/vp split into per-chunk tiles so phase-B reads only wait
            # on the chunk they touch (finer deps -> phase A/B overlap)
            kptc = [
                [sb.tile([128, 1024], BF16, tag=f"kpt{g}_{c}", name=f"kpt{g}_{c}")
                 for c in range(4)]
                for g in range(2)
            ]
            vpc = [sb.tile([128, 4, D], BF16, tag=f"vp{c}", name=f"vp{c}")
                   for c in range(8)]
            attnT = [
                sb.tile([128, QC], F32R, tag=f"attnT{g}", name=f"attnT{g}")
                for g in range(2)
            ]

            with (
                tc.tile_pool(name="upool", bufs=UB) as upool,
                tc.tile_pool(name="tails", bufs=3) as tails,
            ):

                def emit_iter():
                    # ---- phase A: projections ----
                    # QpT: [dv-chunk g 128, q 1024]
                    with tc.tile_pool(name="psQ", bufs=2, space="PSUM") as psQ:
                        for g in range(2):
                            qp_ps = psQ.tile([128, QC], F32, tag="qp_ps")
                            for qb in range(2):
                                for dc in range(2):
                                    nc.tensor.matmul(
                                        qp_ps[:, 512 * qb : 512 * qb + 512],
                                        wqt[dc][:, 128 * g : 128 * g + 128],
                                        qt[dc][:, 512 * qb : 512 * qb + 512],
                                        start=(dc == 0),
                                        stop=(dc == 1),
                                    )
                            if trivial_affine:
                                if ACOPY == "dve":
                                    nc.vector.tensor_copy(qpt[g], qp_ps[:, :])
                                else:
                                    nc.scalar.activation(
                                        out=qpt[g], in_=qp_ps[:, :], func=AF.Copy
                                    )
                            else:
                                nc.vector.tensor_scalar(
                                    out=qpt[g], in0=qp_ps[:, :],
                                    scalar1=vp_[g][:, 0:1], scalar2=None,
                                    op0=ALU.add,
                                )
                    # KpT (K bias dropped: softmax-invariant per query);
                    # two kb chunks share a psum tile -> [128, 1024] copies
                    with tc.tile_pool(name="psK", bufs=2, space="PSUM") as psK:
                        for g in range(2):
                            for kb in range(0, 8, 2):
                                kp_ps = psK.tile([128, 1024], F32, tag="kp_ps")
                                for half in range(2):
                                    for dc in range(2):
                                        nc.tensor.matmul(
                                            kp_ps[:, 512 * half : 512 * half + 512],
                                            wkt[dc][:, 128 * g : 128 * g + 128],
                                            kt[dc][
                                                :,
                                                512 * (kb + half) : 512 * (kb + half) + 512,
                                            ],
                                            start=(dc == 0),
                                            stop=(dc == 1),
                                        )
                                if ACOPY == "dve":
                                    nc.vector.tensor_copy(
                                        kptc[g][kb // 2], kp_ps[:, :]
                                    )
                                else:
                                    nc.scalar.activation(
                                        out=kptc[g][kb // 2],
                                        in_=kp_ps[:, :], func=AF.Copy,
                                    )
                    # Vp: [k-tile 128, dv 256] (V bias folded post-attention);
                    # four k-tiles share a psum tile
                    with tc.tile_pool(name="psV", bufs=3, space="PSUM") as psV:
                        for kt_i in range(0, NJ, 4):
                            vps = psV.tile([128, 4, D], F32, tag="vps")
                            for half in range(4):
                                for dc in range(2):
                                    nc.tensor.matmul(
                                        vps[:, half, :],
                                        kt[dc][
                                            :,
                                            128 * (kt_i + half) : 128 * (kt_i + half) + 128,
                                        ],
                                        wvt[dc][:, :],
                                        start=(dc == 0),
                                        stop=(dc == 1),
                                    )
                            if ACOPY == "dve":
                                nc.vector.tensor_copy(vpc[kt_i // 4], vps)
                            else:
                                nc.scalar.activation(
                                    out=vpc[kt_i // 4], in_=vps, func=AF.Copy,
                                )

                    # ---- phase B: attention ----
                    with (
                        tc.tile_pool(name="scp", bufs=3, space="PSUM") as scp,
                        tc.tile_pool(name="avp", bufs=1, space="PSUM") as avp,
                        tc.tile_pool(name="dnp", bufs=1, space="PSUM") as dnp,
                    ):
                        for qb in range(2):
                            for g in range(2):
                                av_ps = avp.tile([128, 512], F32, tag="av")
                                dn_ps = dnp.tile([128, 512], F32, tag="dn")
                                # av/dn run AVLAG j's behind the score/exp
                                # stream so PE never stalls on one exp
                                uq = []
                                for j in range(NJ + AVLAG):
                                    if j < NJ:
                                        st = [
                                            scp.tile([128, 1024], F32, tag="sc", name="sc")
                                            for _ in range(2)
                                        ]
                                        for hp in range(4):
                                            nc.tensor.matmul(
                                                st[hp // 2][
                                                    :, 512 * (hp % 2) : 512 * (hp % 2) + 512
                                                ],
                                                kptc[g][j // 8][
                                                    32 * hp : 32 * hp + 32,
                                                    128 * (j % 8) : 128 * (j % 8) + 128,
                                                ],
                                                qpt[g][
                                                    32 * hp : 32 * hp + 32,
                                                    512 * qb : 512 * qb + 512,
                                                ],
                                                start=True,
                                                stop=True,
                                                tile_position=(32 * hp, 0),
                                            )
                                        u = [
                                            upool.tile([128, 1024], BF16, tag="u", name="u")
                                            for _ in range(2)
                                        ]
                                        # tile A: ACT native exp; DVE
                                        # cubic every DVEB-th j
                                        if DVEB and j % DVEB == DVEB - 1:
                                            nc.vector._custom_dve(
                                                exp3, out=u[0], in0=st[0][:, :],
                                                in1=c3t, s0=c0, s1=c1, imm2=c2,
                                            )
                                        else:
                                            nc.scalar.activation(
                                                out=u[0], in_=st[0][:, :],
                                                func=AF.Exp, scale=float(SCALE),
                                            )
                                        # tile B: DVE cubic; ACT every ACTB-th j
                                        if ACTB and j % ACTB == ACTB - 1:
                                            nc.scalar.activation(
                                                out=u[1], in_=st[1][:, :],
                                                func=AF.Exp, scale=float(SCALE),
                                            )
                                        else:
                                            nc.vector._custom_dve(
                                                exp3, out=u[1], in0=st[1][:, :],
                                                in1=c3t, s0=c0, s1=c1, imm2=c2,
                                            )
                                        uq.append((u, j))
                                    if (j < NJ and len(uq) > AVLAG) or (
                                        j >= NJ and uq
                                    ):
                                        prev_u, jm = uq.pop(0)
                                        for hp in range(4):
                                            us = prev_u[hp // 2][
                                                :, 512 * (hp % 2) : 512 * (hp % 2) + 512
                                            ]
                                            nc.tensor.matmul(
                                                av_ps[32 * hp : 32 * hp + 32, :],
                                                vpc[jm // 4][
                                                    :, jm % 4,
                                                    128 * g + 32 * hp :
                                                    128 * g + 32 * hp + 32,
                                                ],
                                                us,
                                                start=(jm == 0),
                                                stop=(jm == NJ - 1),
                                                tile_position=(0, 32 * hp),
                                                skip_group_check=True,
                                            )
                                            nc.tensor.matmul(
                                                dn_ps[32 * hp : 32 * hp + 32, :],
                                                ones32[:, :],
                                                us,
                                                start=(jm == 0),
                                                stop=(jm == NJ - 1),
                                                tile_position=(0, 32 * hp),
                                                skip_group_check=True,
                                            )
                                # normalize: attnT = av * (1/den) [+ bv]
                                rden = tails.tile([128, 512], F32, tag="rden")
                                nc.vector.reciprocal_approx_fast(rden, dn_ps[:, :])
                                dst = attnT[g][:, 512 * qb : 512 * qb + 512]
                                nc.vector.tensor_mul(dst, av_ps[:, :], rden)
                                if not trivial_affine:
                                    nc.vector.tensor_scalar(
                                        out=dst, in0=dst, scalar1=vp_[g][:, 1:2],
                                        scalar2=None, op0=ALU.add,
                                    )

                            # ---- tail stats for this q-block (psum-touching
                            # work inline; SBUF-only rsqrt+scale deferred) ----
                            for t4 in range(4):
                                i8 = qb * 4 + t4
                                y_ps = scp.tile([128, 1024], F32, tag="sc", name="y")
                                yp = y_ps[:, 0:256]
                                q0 = 128 * i8
                                for g in range(2):
                                    nc.tensor.matmul(
                                        yp,
                                        attnT[g][:, q0 : q0 + 128],
                                        wot[g][:, :],
                                        start=(g == 0),
                                        stop=(g == 1),
                                    )
                                if trivial_affine:
                                    # fused tail: w = max(2a, a), a = y - mu;
                                    # out = (w - mu_w) * rsqrt(var_w +
                                    #        eps*(var_y + eps)); rsqrt batched
                                    # across all 8 blocks at iteration end
                                    st6 = tails.tile([128, 6], F32, tag="st6")
                                    mv = tails.tile([128, 2], F32, tag="mv")
                                    nc.vector.bn_stats(out=st6, in_=yp)
                                    nc.vector.bn_aggr(out=mv, in_=st6)
                                    # aa = yp - mu on ACT (bias add), -mu from
                                    # Pool: keeps DVE out of this step
                                    nmu = tails.tile([128, 1], F32, tag="nmu")
                                    nc.gpsimd.tensor_scalar(
                                        out=nmu, in0=mv[:, 0:1], scalar1=-1.0,
                                        scalar2=None, op0=ALU.mult,
                                    )
                                    aa = tails.tile([128, D], F32, tag="aa")
                                    nc.scalar.activation(
                                        out=aa, in_=yp, func=AF.Identity,
                                        bias=nmu[:, :],
                                    )
                                    # w = max(2a, a) = a + relu(a) on Pool
                                    zr = tails.tile([128, D], F32, tag="zr")
                                    nc.gpsimd.tensor_scalar(
                                        out=zr, in0=aa, scalar1=0.0,
                                        scalar2=None, op0=ALU.max,
                                    )
                                    nc.gpsimd.tensor_add(wws[:, i8, :], aa, zr)
                                    st6b = tails.tile([128, 6], F32, tag="st6b")
                                    nc.vector.bn_stats(out=st6b, in_=wws[:, i8, :])
                                    nc.vector.bn_aggr(out=mvbs[:, i8, :], in_=st6b)
                                    # d = var_w + eps*(var_y + eps)
                                    ddt = tails.tile([128, 1], F32, tag="ddt")
                                    nc.gpsimd.tensor_scalar(
                                        out=ddt, in0=mv[:, 1:2], scalar1=LN_EPS,
                                        scalar2=LN_EPS, op0=ALU.add, op1=ALU.mult,
                                    )
                                    nc.gpsimd.tensor_add(
                                        dds[:, i8 : i8 + 1], ddt, mvbs[:, i8, 1:2]
                                    )
                                else:
                                    nc.vector.tensor_add(yp, yp, vf_[:, 0, :])
                                    st6 = tails.tile([128, 6], F32, tag="st6")
                                    mv = tails.tile([128, 2], F32, tag="mv")
                                    rs = tails.tile([128, 1], F32, tag="rs")
                                    nc.vector.bn_stats(out=st6, in_=yp)
                                    nc.vector.bn_aggr(out=mv, in_=st6)
                                    nc.scalar.activation(
                                        out=rs, in_=mv[:, 1:2], func=AF.Ln,
                                        bias=epst[:, :],
                                    )
                                    nc.scalar.activation(
                                        out=rs, in_=rs, func=AF.Exp, scale=-0.5
                                    )
                                    h0 = tails.tile([128, D], F32, tag="h0")
                                    nc.vector.tensor_scalar(
                                        out=h0, in0=yp, scalar1=mv[:, 0:1],
                                        scalar2=rs, op0=ALU.subtract, op1=ALU.mult,
                                    )
                                    nc.vector.tensor_mul(h0, h0, vf_[:, 1, :])
                                    nc.vector.tensor_add(h0, h0, vf_[:, 2, :])
                                    zr = tails.tile([128, D], F32, tag="zr")
                                    nc.vector.tensor_scalar_max(zr, h0, 0.0)
                                    z = tails.tile([128, D], F32, tag="z")
                                    nc.vector.tensor_add(z, h0, zr)
                                    st6b = tails.tile([128, 6], F32, tag="st6b")
                                    mvb = tails.tile([128, 2], F32, tag="mvb")
                                    rsb = tails.tile([128, 1], F32, tag="rsb")
                                    nc.vector.bn_stats(out=st6b, in_=z)
                                    nc.vector.bn_aggr(out=mvb, in_=st6b)
                                    nc.scalar.activation(
                                        out=rsb, in_=mvb[:, 1:2], func=AF.Ln,
                                        bias=epst[:, :],
                                    )
                                    nc.scalar.activation(
                                        out=rsb, in_=rsb, func=AF.Exp, scale=-0.5
                                    )
                                    ot = tails.tile([128, D], F32, tag="ot")
                                    nc.vector.tensor_scalar(
                                        out=ot, in0=z, scalar1=mvb[:, 0:1],
                                        scalar2=rsb, op0=ALU.subtract, op1=ALU.mult,
                                    )
                                    nc.vector.tensor_mul(ot, ot, vf_[:, 3, :])
                                    nc.vector.tensor_add(ot, ot, vf_[:, 4, :])
                                    nc.sync.dma_start(
                                        out=out[q0 : q0 + 128, :], in_=ot
                                    )

                        if trivial_affine:
                            # batched rsqrt = exp(-0.5*ln(d)) for all 8 tail
                            # blocks in one Ln+Exp pair (SBUF-only epilogue)
                            rsa = tails.tile([128, 8], F32, tag="rsa")
                            nc.scalar.activation(out=rsa, in_=dds, func=AF.Ln)
                            nc.scalar.activation(
                                out=rsa, in_=rsa, func=AF.Exp, scale=-0.5
                            )
                            for i8 in range(8):
                                ot = tails.tile([128, D], F32, tag="ot")
                                nc.gpsimd.tensor_scalar(
                                    out=ot, in0=wws[:, i8, :],
                                    scalar1=mvbs[:, i8, 0:1],
                                    scalar2=rsa[:, i8 : i8 + 1],
                                    op0=ALU.subtract, op1=ALU.mult,
                                )
                                nc.sync.dma_start(
                                    out=out[128 * i8 : 128 * i8 + 128, :], in_=ot
                                )

                if repeat == 1:
                    emit_iter()
                elif os.environ.get("PYUNROLL") == "1":
                    # python-unrolled repeat: lets the no-exec TimelineSim
                    # measure steady-state (it cannot follow reg-mode loops)
                    for _ in range(repeat):
                        emit_iter()
                else:
                    with tc.For_i(0, repeat):
                        emit_iter()

    nc.compile()
    return nc


_KERNEL_CACHE = {}


def _get_kernel(trivial_affine, repeat=1):
    key = (
        bool(trivial_affine), int(repeat),
        os.environ.get("ACTB", "0"), os.environ.get("ACOPY", "act"),
        os.environ.get("UB", ""), os.environ.get("AVLAG", "2"),
        os.environ.get("DVEB", "0"),
    )
    if key not in _KERNEL_CACHE:
        _KERNEL_CACHE[key] = _build_kernel(key[0], key[1])
    return _KERNEL_CACHE[key]


def _prep(Q, K, Wq, bq, Wk, bk, Wv, bv, Wo, bo, g0, beta0, g1, beta1):
    """Shared input prep: returns (trivial_affine, in_maps)."""
    Q = np.asarray(Q, dtype=np.float32)
    K = np.asarray(K, dtype=np.float32)
    Wq = np.asarray(Wq, dtype=np.float32)
    Wk = np.asarray(Wk, dtype=np.float32)
    Wv = np.asarray(Wv, dtype=np.float32)
    Wo = np.asarray(Wo, dtype=np.float32)
    bq, bv, bo, g0, beta0, g1, beta1 = [
        np.asarray(v, dtype=np.float32)
        for v in (bq, bv, bo, g0, beta0, g1, beta1)
    ]

    trivial = bool(
        not bq.any() and not bv.any() and not bo.any()
        and not beta0.any() and not beta1.any()
        and np.all(g0 == 1.0) and np.all(g1 == 1.0)
    )

    wqTn = np.ascontiguousarray(Wq.T)
    wkTn = np.ascontiguousarray(Wk.T)
    wvTn = np.ascontiguousarray(Wv.T)
    woTn = np.ascontiguousarray(Wo.T)
    vecsP = np.stack([bq, bv], axis=1).astype(np.float32)  # [D, 2]
    vecsF = np.stack([bo, g0, beta0, g1, beta1], axis=0).astype(np.float32)

    kTb = [np.ascontiguousarray(K[b].T) for b in range(B)]
    in_maps = []
    for c in range(NCORES):
        b, qc = divmod(c, NCORES // B)
        in_maps.append(
            {
                "qT": np.ascontiguousarray(Q[b, QC * qc : QC * qc + QC, :].T),
                "kT": kTb[b],
                "wqT": wqTn,
                "wkT": wkTn,
                "wvT": wvTn,
                "woT": woTn,
                "vecsP": vecsP,
                "vecsF": vecsF,
            }
        )
    return trivial, in_maps


def _gather(res):
    outp = np.empty((B, NQ, D), dtype=np.float32)
    for c in range(NCORES):
        b, qc = divmod(c, NCORES // B)
        outp[b, QC * qc : QC * qc + QC, :] = res.results[c]["out"]
    return outp


def kernel(**inputs):
    trivial, in_maps = _prep(**inputs)
    nc = _get_kernel(trivial)
    res = run_bass_kernel_spmd(nc, in_maps, list(range(NCORES)))
    return _gather(res)


# revision 39
# speedup vs baseline: 1.0171x; 1.0171x over previous
"""Fused multi-head attention block (QKV proj + softmax attention + out proj
+ LN + relu-residual + LN) for Trainium2, SPMD across 8 NeuronCores.

Problem shapes (hardcoded): B=2, NQ=NK=4096, D=256, H=8, DH=32.

Sharding: sequence-parallel over (batch, query-chunk): core c handles batch
c//4, query rows [1024*(c%4), 1024*(c%4+1)). Each core reads the K rows of
its batch and computes its query chunk end-to-end. No collectives.

Engine plan (HW-calibrated: tile-packed matmuls at distinct tile_positions
execute CONCURRENTLY on the PE quads, so the packed scores/attn/den matmuls
cost ~1/4 of their serial instruction time; output-partition-narrow matmuls
are fine when packed; weight reloads for full [128,128] stationary tiles
are NOT free, so the attn@V stays in the V-stationary orientation):
  PE    : projections; scoresT (4x row-packed K=32); attn@V + denominator
          (4x col-packed each, [32,512] out, V/ones stationary); out-proj.
  ACT   : exp of score tile A (native Exp, one [128,1024] instr/j) and of
          tile B every ACTB-th j; phase-A PSUM->SBUF copies; a = y - mu
          (bias-add); one batched Ln+Exp rsqrt for all 8 tail blocks per
          iteration (kills the per-tail act-table thrash, ~17 table loads).
  DVE   : exp of score tile B via 1-pass custom cubic, bn_stats/aggr,
          softmax reciprocal + normalize.
  Pool  : SBUF-only tail elementwise: -mu, relu/add, d calc, final scale.

Pipelining: av/dn run AVLAG j's behind the score/exp stream (u-tile ring
depth 2*(AVLAG+1)) so PE never stalls on a single exp's jitter. kpt/vp are
split into per-chunk tiles so phase-B reads only wait on the chunk they
touch (finer deps let phase A overlap the previous iteration's tail). The
tail keeps only PSUM-touching work inline per q-block; the SBUF-only
rsqrt+scale+store runs once at iteration end, so the next iteration's
phase A isn't gated behind it by PSUM pool aliasing.

Fused tail: with t = (y-mu)*rs (LN0) and z = t + relu(t) (= rs*w where
w = max(2a, a), a = y-mu), LN1(z) = (w - mean(w)) * rsqrt(var(w) +
eps*(var(y)+eps)) -- LN0's rs never needs to be computed.

Knobs (env): ACTB=0 (ACT additionally takes score tile B every ACTB-th j),
ACOPY=split (phase-A copy engine: Kp/Vp evacs alternate ACT/DVE), AVLAG=2, UB (u ring, default from AVLAG).
"""

import os

import numpy as np

import concourse.bass as bass
import concourse.mybir as mybir
import concourse.tile as tile
from concourse import bacc
from concourse.bass_utils import run_bass_kernel_spmd

F32 = mybir.dt.float32
F32R = mybir.dt.float32r
BF16 = mybir.dt.bfloat16
AF = mybir.ActivationFunctionType
ALU = mybir.AluOpType

B, NQ, NK = 2, 4096, 4096
D = 256
H = 8
DH = 32
LN_EPS = 1e-5
NCORES = 8
QC = (B * NQ) // NCORES  # 1024 query rows per core
SCALE = 1.0 / np.sqrt(np.float32(DH))
NJ = NK // 128  # 32 k-tiles

_DVE_OPS = {}


def _register_dve_ops():
    """Runtime-register the custom DVE ops used by this kernel."""
    if _DVE_OPS:
        return _DVE_OPS
    import concourse.dve_ops as dve_ops
    from concourse.dve_spec import (
        C0, C1, C2, C3, Spec, Src0, _spill_c3_to_src1, lower,
    )
    from concourse.dve_uop import DveOpSpec

    def _mk(name, spec, rd1_en):
        for op in dve_ops.OPS:
            if op.name == name:
                return op
        row = dve_ops._CUSTOM_DVE_ROW_BASE + len(dve_ops.OPS)
        shas = {}
        for ver in ("v3", "v4"):
            tmp = DveOpSpec(name=name, opcode=row, uops=lower(spec, ver=ver),
                            rd1_en=rd1_en)
            shas[ver] = tmp.sha(ver)
        op = dve_ops.DveOp(name, spec, subdim=False, uops_sha=shas)
        dve_ops.OPS.append(op)
        dve_ops.CUSTOM_DVE_SPECS[op.name] = op.spec
        dve_ops._SUB_OPCODE_FOR_NAME[op.name] = row
        return op

    # cubic exp: out = ((c3*x + c2)*x + c1)*x + c0, c3 rides in1 ([P,1])
    def _exp3_ref(in0, in1, c0, c1, c2):
        c3 = in1[:, :1]
        x = in0.astype(np.float32)
        return ((c3 * x + c2) * x + c1) * x + c0

    exp3 = _mk(
        "EXP3_ANT",
        Spec(
            body=_spill_c3_to_src1(((C3 * Src0 + C2) * Src0 + C1) * Src0 + C0),
            reference=_exp3_ref,
        ),
        rd1_en=True,
    )
    _DVE_OPS["exp3"] = exp3
    return _DVE_OPS


def _fit_exp_cubic(scale, hi_raw):
    """Chebyshev-node cubic fit of e^(scale*x) for x in [-hi_raw, hi_raw]
    (raw, unscaled scores). Returns (c0, c1, c2, c3)."""
    t = np.cos(np.linspace(0, np.pi, 20001))
    xc = hi_raw * t
    yc = np.exp(np.float64(scale) * xc)
    c = np.polyfit(xc, yc, 3)
    return tuple(float(v) for v in c[::-1])


def _build_kernel(trivial_affine, repeat=1):
    """Build the SPMD Bass program. trivial_affine: all biases zero, all LN
    gammas one, betas zero (true for this problem's setup_inputs)."""
    ops = _register_dve_ops()
    exp3 = ops["exp3"]
    c0, c1, c2, c3 = _fit_exp_cubic(SCALE, 4.6)
    # ACT additionally takes score tile B every ACTB-th j (0 = never)
    ACTB = int(os.environ.get("ACTB", "0"))
    # phase-A PSUM->SBUF copy engine: act | dve
    ACOPY = os.environ.get("ACOPY", "split")
    # DVE additionally takes score tile A every DVEB-th j (0 = never)
    DVEB = int(os.environ.get("DVEB", "0"))
    AVLAG = int(os.environ.get("AVLAG", "2"))
    UB = int(os.environ.get("UB", str(2 * (int(os.environ.get("AVLAG", "2")) + 1))))

    nc = bacc.Bacc("TRN2", target_bir_lowering=False)

    # ---- dram i/o ----
    qT = nc.dram_tensor("qT", [D, QC], F32R, kind="ExternalInput")
    kT = nc.dram_tensor("kT", [D, NK], F32R, kind="ExternalInput")
    wqT = nc.dram_tensor("wqT", [D, D], F32R, kind="ExternalInput")
    wkT = nc.dram_tensor("wkT", [D, D], F32R, kind="ExternalInput")
    wvT = nc.dram_tensor("wvT", [D, D], F32R, kind="ExternalInput")
    woT = nc.dram_tensor("woT", [D, D], F32R, kind="ExternalInput")
    # vecsP[d, i]: per-partition-use vectors; col 0=bq, 1=bv
    vecsP = nc.dram_tensor("vecsP", [D, 2], F32, kind="ExternalInput")
    # vecsF[i, d]: free-dim-use vectors; row 0=bo 1=g0 2=beta0 3=g1 4=beta1
    vecsF = nc.dram_tensor("vecsF", [5, D], F32, kind="ExternalInput")
    out = nc.dram_tensor("out", [QC, D], F32, kind="ExternalOutput")

    with tile.TileContext(nc) as tc:
        with tc.tile_pool(name="sb", bufs=1) as sb:
            # ---- load inputs (q/k chunked so phase A streams behind) ----
            qt = [sb.tile([128, QC], F32R, tag=f"qt{i}", name=f"qt{i}") for i in range(2)]
            kt = [sb.tile([128, NK], F32R, tag=f"kt{i}", name=f"kt{i}") for i in range(2)]
            wqt = [sb.tile([128, D], F32R, tag=f"wqt{i}", name=f"wqt{i}") for i in range(2)]
            wkt = [sb.tile([128, D], F32R, tag=f"wkt{i}", name=f"wkt{i}") for i in range(2)]
            wvt = [sb.tile([128, D], F32R, tag=f"wvt{i}", name=f"wvt{i}") for i in range(2)]
            wot = [sb.tile([128, D], F32R, tag=f"wot{i}", name=f"wot{i}") for i in range(2)]
            ones32 = sb.tile([128, 32], BF16)
            c3t = sb.tile([128, 1], F32)
            epst = sb.tile([128, 1], F32)
            vp_ = [sb.tile([128, 2], F32, tag=f"vp_{i}", name=f"vp_{i}") for i in range(2)]
            vf_ = sb.tile([128, 5, D], F32) if not trivial_affine else None
            # per-tail-block persistent state (written each iter, read at end)
            wws = sb.tile([128, 8, D], F32)
            mvbs = sb.tile([128, 8, 2], F32)
            dds = sb.tile([128, 8], F32)
            for i in range(2):
                nc.sync.dma_start(out=wqt[i], in_=wqT[128 * i : 128 * i + 128, :])
                nc.sync.dma_start(out=wkt[i], in_=wkT[128 * i : 128 * i + 128, :])
                nc.sync.dma_start(out=wvt[i], in_=wvT[128 * i : 128 * i + 128, :])
                nc.sync.dma_start(out=wot[i], in_=woT[128 * i : 128 * i + 128, :])
                nc.sync.dma_start(out=qt[i], in_=qT[128 * i : 128 * i + 128, :])
                for ck in range(4):
                    nc.sync.dma_start(
                        out=kt[i][:, 1024 * ck : 1024 * ck + 1024],
                        in_=kT[128 * i : 128 * i + 128,
                              1024 * ck : 1024 * ck + 1024],
                    )
                nc.sync.dma_start(out=vp_[i], in_=vecsP[128 * i : 128 * i + 128, :])
            nc.vector.memset(ones32, 1.0)
            if vf_ is not None:
                nc.gpsimd.dma_start(
                    out=vf_, in_=vecsF[:, :].unsqueeze(0).broadcast_to([128, 5, D])
                )
            nc.vector.memset(c3t, c3)
            nc.vector.memset(epst, LN_EPS)
            # warm the ACT exp/ln table set while input DMAs stream
            warmt = sb.tile([128, 1], F32)
            nc.scalar.activation(out=warmt, in_=epst, func=AF.Exp)
            nc.scalar.activation(out=warmt, in_=warmt, func=AF.Ln)

            qpt = [sb.tile([128, QC], BF16, tag=f"qpt{g}", name=f"qpt{g}")
                   for g in range(2)]
            # kpt/vp split into per-chunk tiles so phase-B reads only wait
            # on the chunk they touch (finer deps -> phase A/B overlap)
            kptc = [
                [sb.tile([128, 1024], BF16, tag=f"kpt{g}_{c}", name=f"kpt{g}_{c}")
                 for c in range(4)]
                for g in range(2)
            ]
            vpc = [sb.tile([128, 4, D], BF16, tag=f"vp{c}", name=f"vp{c}")
                   for c in range(8)]
            attnT = [
                sb.tile([128, QC], F32R, tag=f"attnT{g}", name=f"attnT{g}")
                for g in range(2)
            ]

            with (
                tc.tile_pool(name="upool", bufs=UB) as upool,
                tc.tile_pool(name="tails", bufs=3) as tails,
            ):

                def emit_iter():
                    # ---- phase A: projections ----
                    # QpT: [dv-chunk g 128, q 1024]
                    with tc.tile_pool(name="psQ", bufs=2, space="PSUM") as psQ:
                        for g in range(2):
                            qp_ps = psQ.tile([128, QC], F32, tag="qp_ps")
                            for qb in range(2):
                                for dc in range(2):
                                    nc.tensor.matmul(
                                        qp_ps[:, 512 * qb : 512 * qb + 512],
                                        wqt[dc][:, 128 * g : 128 * g + 128],
                                        qt[dc][:, 512 * qb : 512 * qb + 512],
                                        start=(dc == 0),
                                        stop=(dc == 1),
                                    )
                            if trivial_affine:
                                if ACOPY == "dve":
                                    nc.vector.tensor_copy(qpt[g], qp_ps[:, :])
                                else:
                                    nc.scalar.activation(
                                        out=qpt[g], in_=qp_ps[:, :], func=AF.Copy
                                    )
                            else:
                                nc.vector.tensor_scalar(
                                    out=qpt[g], in0=qp_ps[:, :],
                                    scalar1=vp_[g][:, 0:1], scalar2=None,
                                    op0=ALU.add,
                                )
                    # KpT (K bias dropped: softmax-invariant per query);
                    # two kb chunks share a psum tile -> [128, 1024] copies
                    with tc.tile_pool(name="psK", bufs=2, space="PSUM") as psK:
                        for g in range(2):
                            for kb in range(0, 8, 2):
                                kp_ps = psK.tile([128, 1024], F32, tag="kp_ps")
                                for half in range(2):
                                    for dc in range(2):
                                        nc.tensor.matmul(
                                            kp_ps[:, 512 * half : 512 * half + 512],
                                            wkt[dc][:, 128 * g : 128 * g + 128],
                                            kt[dc][
                                                :,
                                                512 * (kb + half) : 512 * (kb + half) + 512,
                                            ],
                                            start=(dc == 0),
                                            stop=(dc == 1),
                                        )
                                if ACOPY == "dve" or (
                                    ACOPY == "split" and (kb // 2) % 2 == 1
                                ):
                                    nc.vector.tensor_copy(
                                        kptc[g][kb // 2], kp_ps[:, :]
                                    )
                                else:
                                    nc.scalar.activation(
                                        out=kptc[g][kb // 2],
                                        in_=kp_ps[:, :], func=AF.Copy,
                                    )
                    # Vp: [k-tile 128, dv 256] (V bias folded post-attention);
                    # four k-tiles share a psum tile
                    with tc.tile_pool(name="psV", bufs=3, space="PSUM") as psV:
                        for kt_i in range(0, NJ, 4):
                            vps = psV.tile([128, 4, D], F32, tag="vps")
                            for half in range(4):
                                for dc in range(2):
                                    nc.tensor.matmul(
                                        vps[:, half, :],
                                        kt[dc][
                                            :,
                                            128 * (kt_i + half) : 128 * (kt_i + half) + 128,
                                        ],
                                        wvt[dc][:, :],
                                        start=(dc == 0),
                                        stop=(dc == 1),
                                    )
                            if ACOPY == "dve" or (
                                ACOPY == "split" and (kt_i // 4) % 2 == 0
                            ):
                                nc.vector.tensor_copy(vpc[kt_i // 4], vps)
                            else:
                                nc.scalar.activation(
                                    out=vpc[kt_i // 4], in_=vps, func=AF.Copy,
                                )

                    # ---- phase B: attention ----
                    with (
                        tc.tile_pool(name="scp", bufs=3, space="PSUM") as scp,
                        tc.tile_pool(name="avp", bufs=1, space="PSUM") as avp,
                        tc.tile_pool(name="dnp", bufs=1, space="PSUM") as dnp,
                    ):
                        for qb in range(2):
                            for g in range(2):
                                av_ps = avp.tile([128, 512], F32, tag="av")
                                dn_ps = dnp.tile([128, 512], F32, tag="dn")
                                # av/dn run AVLAG j's behind the score/exp
                                # stream so PE never stalls on one exp
                                uq = []
                                for j in range(NJ + AVLAG):
                                    if j < NJ:
                                        st = [
                                            scp.tile([128, 1024], F32, tag="sc", name="sc")
                                            for _ in range(2)
                                        ]
                                        for hp in range(4):
                                            nc.tensor.matmul(
                                                st[hp // 2][
                                                    :, 512 * (hp % 2) : 512 * (hp % 2) + 512
                                                ],
                                                kptc[g][j // 8][
                                                    32 * hp : 32 * hp + 32,
                                                    128 * (j % 8) : 128 * (j % 8) + 128,
                                                ],
                                                qpt[g][
                                                    32 * hp : 32 * hp + 32,
                                                    512 * qb : 512 * qb + 512,
                                                ],
                                                start=True,
                                                stop=True,
                                                tile_position=(32 * hp, 0),
                                            )
                                        u = [
                                            upool.tile([128, 1024], BF16, tag="u", name="u")
                                            for _ in range(2)
                                        ]
                                        # tile A: ACT native exp; DVE
                                        # cubic every DVEB-th j
                                        if DVEB and j % DVEB == DVEB - 1:
                                            nc.vector._custom_dve(
                                                exp3, out=u[0], in0=st[0][:, :],
                                                in1=c3t, s0=c0, s1=c1, imm2=c2,
                                            )
                                        else:
                                            nc.scalar.activation(
                                                out=u[0], in_=st[0][:, :],
                                                func=AF.Exp, scale=float(SCALE),
                                            )
                                        # tile B: DVE cubic; ACT every ACTB-th j
                                        if ACTB and j % ACTB == ACTB - 1:
                                            nc.scalar.activation(
                                                out=u[1], in_=st[1][:, :],
                                                func=AF.Exp, scale=float(SCALE),
                                            )
                                        else:
                                            nc.vector._custom_dve(
                                                exp3, out=u[1], in0=st[1][:, :],
                                                in1=c3t, s0=c0, s1=c1, imm2=c2,
                                            )
                                        uq.append((u, j))
                                    if (j < NJ and len(uq) > AVLAG) or (
                                        j >= NJ and uq
                                    ):
                                        prev_u, jm = uq.pop(0)
                                        for hp in range(4):
                                            us = prev_u[hp // 2][
                                                :, 512 * (hp % 2) : 512 * (hp % 2) + 512
                                            ]
                                            nc.tensor.matmul(
                                                av_ps[32 * hp : 32 * hp + 32, :],
                                                vpc[jm // 4][
                                                    :, jm % 4,
                                                    128 * g + 32 * hp :
                                                    128 * g + 32 * hp + 32,
                                                ],
                                                us,
                                                start=(jm == 0),
                                                stop=(jm == NJ - 1),
                                                tile_position=(0, 32 * hp),
                                                skip_group_check=True,
                                            )
                                            nc.tensor.matmul(
                                                dn_ps[32 * hp : 32 * hp + 32, :],
                                                ones32[:, :],
                                                us,
                                                start=(jm == 0),
                                                stop=(jm == NJ - 1),
                                                tile_position=(0, 32 * hp),
                                                skip_group_check=True,
                                            )
                                # normalize: attnT = av * (1/den) [+ bv]
                                rden = tails.tile([128, 512], F32, tag="rden")
                                nc.vector.reciprocal_approx_fast(rden, dn_ps[:, :])
                                dst = attnT[g][:, 512 * qb : 512 * qb + 512]
                                nc.vector.tensor_mul(dst, av_ps[:, :], rden)
                                if not trivial_affine:
                                    nc.vector.tensor_scalar(
                                        out=dst, in0=dst, scalar1=vp_[g][:, 1:2],
                                        scalar2=None, op0=ALU.add,
                                    )

                            # ---- tail stats for this q-block (psum-touching
                            # work inline; SBUF-only rsqrt+scale deferred) ----
                            for t4 in range(4):
                                i8 = qb * 4 + t4
                                y_ps = scp.tile([128, 1024], F32, tag="sc", name="y")
                                yp = y_ps[:, 0:256]
                                q0 = 128 * i8
                                for g in range(2):
                                    nc.tensor.matmul(
                                        yp,
                                        attnT[g][:, q0 : q0 + 128],
                                        wot[g][:, :],
                                        start=(g == 0),
                                        stop=(g == 1),
                                    )
                                if trivial_affine:
                                    # fused tail: w = max(2a, a), a = y - mu;
                                    # out = (w - mu_w) * rsqrt(var_w +
                                    #        eps*(var_y + eps)); rsqrt batched
                                    # across all 8 blocks at iteration end
                                    st6 = tails.tile([128, 6], F32, tag="st6")
                                    mv = tails.tile([128, 2], F32, tag="mv")
                                    nc.vector.bn_stats(out=st6, in_=yp)
                                    nc.vector.bn_aggr(out=mv, in_=st6)
                                    # aa = yp - mu on ACT (bias add), -mu from
                                    # Pool: keeps DVE out of this step
                                    nmu = tails.tile([128, 1], F32, tag="nmu")
                                    nc.gpsimd.tensor_scalar(
                                        out=nmu, in0=mv[:, 0:1], scalar1=-1.0,
                                        scalar2=None, op0=ALU.mult,
                                    )
                                    aa = tails.tile([128, D], F32, tag="aa")
                                    nc.scalar.activation(
                                        out=aa, in_=yp, func=AF.Identity,
                                        bias=nmu[:, :],
                                    )
                                    # w = max(2a, a) = a + relu(a) on Pool
                                    zr = tails.tile([128, D], F32, tag="zr")
                                    nc.gpsimd.tensor_scalar(
                                        out=zr, in0=aa, scalar1=0.0,
                                        scalar2=None, op0=ALU.max,
                                    )
                                    nc.gpsimd.tensor_add(wws[:, i8, :], aa, zr)
                                    st6b = tails.tile([128, 6], F32, tag="st6b")
                                    nc.vector.bn_stats(out=st6b, in_=wws[:, i8, :])
                                    nc.vector.bn_aggr(out=mvbs[:, i8, :], in_=st6b)
                                    # d = var_w + eps*(var_y + eps)
                                    ddt = tails.tile([128, 1], F32, tag="ddt")
                                    nc.gpsimd.tensor_scalar(
                                        out=ddt, in0=mv[:, 1:2], scalar1=LN_EPS,
                                        scalar2=LN_EPS, op0=ALU.add, op1=ALU.mult,
                                    )
                                    nc.gpsimd.tensor_add(
                                        dds[:, i8 : i8 + 1], ddt, mvbs[:, i8, 1:2]
                                    )
                                else:
                                    nc.vector.tensor_add(yp, yp, vf_[:, 0, :])
                                    st6 = tails.tile([128, 6], F32, tag="st6")
                                    mv = tails.tile([128, 2], F32, tag="mv")
                                    rs = tails.tile([128, 1], F32, tag="rs")
                                    nc.vector.bn_stats(out=st6, in_=yp)
                                    nc.vector.bn_aggr(out=mv, in_=st6)
                                    nc.scalar.activation(
                                        out=rs, in_=mv[:, 1:2], func=AF.Ln,
                                        bias=epst[:, :],
                                    )
                                    nc.scalar.activation(
                                        out=rs, in_=rs, func=AF.Exp, scale=-0.5
                                    )
                                    h0 = tails.tile([128, D], F32, tag="h0")
                                    nc.vector.tensor_scalar(
                                        out=h0, in0=yp, scalar1=mv[:, 0:1],
                                        scalar2=rs, op0=ALU.subtract, op1=ALU.mult,
                                    )
                                    nc.vector.tensor_mul(h0, h0, vf_[:, 1, :])
                                    nc.vector.tensor_add(h0, h0, vf_[:, 2, :])
                                    zr = tails.tile([128, D], F32, tag="zr")
                                    nc.vector.tensor_scalar_max(zr, h0, 0.0)
                                    z = tails.tile([128, D], F32, tag="z")
                                    nc.vector.tensor_add(z, h0, zr)
                                    st6b = tails.tile([128, 6], F32, tag="st6b")
                                    mvb = tails.tile([128, 2], F32, tag="mvb")
                                    rsb = tails.tile([128, 1], F32, tag="rsb")
                                    nc.vector.bn_stats(out=st6b, in_=z)
                                    nc.vector.bn_aggr(out=mvb, in_=st6b)
                                    nc.scalar.activation(
                                        out=rsb, in_=mvb[:, 1:2], func=AF.Ln,
                                        bias=epst[:, :],
                                    )
                                    nc.scalar.activation(
                                        out=rsb, in_=rsb, func=AF.Exp, scale=-0.5
                                    )
                                    ot = tails.tile([128, D], F32, tag="ot")
                                    nc.vector.tensor_scalar(
                                        out=ot, in0=z, scalar1=mvb[:, 0:1],
                                        scalar2=rsb, op0=ALU.subtract, op1=ALU.mult,
                                    )
                                    nc.vector.tensor_mul(ot, ot, vf_[:, 3, :])
                                    nc.vector.tensor_add(ot, ot, vf_[:, 4, :])
                                    nc.sync.dma_start(
                                        out=out[q0 : q0 + 128, :], in_=ot
                                    )

                        if trivial_affine:
                            # batched rsqrt = exp(-0.5*ln(d)) for all 8 tail
                            # blocks in one Ln+Exp pair (SBUF-only epilogue)
                            rsa = tails.tile([128, 8], F32, tag="rsa")
                            nc.scalar.activation(out=rsa, in_=dds, func=AF.Ln)
                            nc.scalar.activation(
                                out=rsa, in_=rsa, func=AF.Exp, scale=-0.5
                            )
                            for i8 in range(8):
                                ot = tails.tile([128, D], F32, tag="ot")
                                nc.gpsimd.tensor_scalar(
                                    out=ot, in0=wws[:, i8, :],
                                    scalar1=mvbs[:, i8, 0:1],
                                    scalar2=rsa[:, i8 : i8 + 1],
                                    op0=ALU.subtract, op1=ALU.mult,
                                )
                                nc.sync.dma_start(
                                    out=out[128 * i8 : 128 * i8 + 128, :], in_=ot
                                )

                if repeat == 1:
                    emit_iter()
                elif os.environ.get("PYUNROLL") == "1":
                    # python-unrolled repeat: lets the no-exec TimelineSim
                    # measure steady-state (it cannot follow reg-mode loops)
                    for _ in range(repeat):
                        emit_iter()
                else:
                    with tc.For_i(0, repeat):
                        emit_iter()

    nc.compile()
    return nc


_KERNEL_CACHE = {}


def _get_kernel(trivial_affine, repeat=1):
    key = (
        bool(trivial_affine), int(repeat),
        os.environ.get("ACTB", "0"), os.environ.get("ACOPY", "split"),
        os.environ.get("UB", ""), os.environ.get("AVLAG", "2"),
        os.environ.get("DVEB", "0"),
    )
    if key not in _KERNEL_CACHE:
        _KERNEL_CACHE[key] = _build_kernel(key[0], key[1])
    return _KERNEL_CACHE[key]


def _prep(Q, K, Wq, bq, Wk, bk, Wv, bv, Wo, bo, g0, beta0, g1, beta1):
    """Shared input prep: returns (trivial_affine, in_maps)."""
    Q = np.asarray(Q, dtype=np.float32)
    K = np.asarray(K, dtype=np.float32)
    Wq = np.asarray(Wq, dtype=np.float32)
    Wk = np.asarray(Wk, dtype=np.float32)
    Wv = np.asarray(Wv, dtype=np.float32)
    Wo = np.asarray(Wo, dtype=np.float32)
    bq, bv, bo, g0, beta0, g1, beta1 = [
        np.asarray(v, dtype=np.float32)
        for v in (bq, bv, bo, g0, beta0, g1, beta1)
    ]

    trivial = bool(
        not bq.any() and not bv.any() and not bo.any()
        and not beta0.any() and not beta1.any()
        and np.all(g0 == 1.0) and np.all(g1 == 1.0)
    )

    wqTn = np.ascontiguousarray(Wq.T)
    wkTn = np.ascontiguousarray(Wk.T)
    wvTn = np.ascontiguousarray(Wv.T)
    woTn = np.ascontiguousarray(Wo.T)
    vecsP = np.stack([bq, bv], axis=1).astype(np.float32)  # [D, 2]
    vecsF = np.stack([bo, g0, beta0, g1, beta1], axis=0).astype(np.float32)

    kTb = [np.ascontiguousarray(K[b].T) for b in range(B)]
    in_maps = []
    for c in range(NCORES):
        b, qc = divmod(c, NCORES // B)
        in_maps.append(
            {
                "qT": np.ascontiguousarray(Q[b, QC * qc : QC * qc + QC, :].T),
                "kT": kTb[b],
                "wqT": wqTn,
                "wkT": wkTn,
                "wvT": wvTn,
                "woT": woTn,
                "vecsP": vecsP,
                "vecsF": vecsF,
            }
        )
    return trivial, in_maps


def _gather(res):
    outp = np.empty((B, NQ, D), dtype=np.float32)
    for c in range(NCORES):
        b, qc = divmod(c, NCORES // B)
        outp[b, QC * qc : QC * qc + QC, :] = res.results[c]["out"]
    return outp


def kernel(**inputs):
    trivial, in_maps = _prep(**inputs)
    nc = _get_kernel(trivial)
    res = run_bass_kernel_spmd(nc, in_maps, list(range(NCORES)))
    return _gather(res)


# revision 40
# speedup vs baseline: 1.0256x; 1.0083x over previous
"""Fused multi-head attention block (QKV proj + softmax attention + out proj
+ LN + relu-residual + LN) for Trainium2, SPMD across 8 NeuronCores.

Problem shapes (hardcoded): B=2, NQ=NK=4096, D=256, H=8, DH=32.

Sharding: sequence-parallel over (batch, query-chunk): core c handles batch
c//4, query rows [1024*(c%4), 1024*(c%4+1)). Each core reads the K rows of
its batch and computes its query chunk end-to-end. No collectives.

Engine plan (HW-calibrated: tile-packed matmuls at distinct tile_positions
execute CONCURRENTLY on the PE quads, so the packed scores/attn/den matmuls
cost ~1/4 of their serial instruction time; output-partition-narrow matmuls
are fine when packed; weight reloads for full [128,128] stationary tiles
are NOT free, so the attn@V stays in the V-stationary orientation):
  PE    : projections; scoresT (4x row-packed K=32); attn@V + denominator
          (4x col-packed each, [32,512] out, V/ones stationary); out-proj.
  ACT   : exp of score tile A (native Exp, one [128,1024] instr/j) and of
          tile B every ACTB-th j; phase-A PSUM->SBUF copies; a = y - mu
          (bias-add); one batched Ln+Exp rsqrt for all 8 tail blocks per
          iteration (kills the per-tail act-table thrash, ~17 table loads).
  DVE   : exp of score tile B via 1-pass custom cubic, bn_stats/aggr,
          softmax reciprocal + normalize.
  Pool  : SBUF-only tail elementwise: -mu, relu/add, d calc, final scale.

Pipelining: av/dn run AVLAG j's behind the score/exp stream (u-tile ring
depth 2*(AVLAG+1)) so PE never stalls on a single exp's jitter. kpt/vp are
split into per-chunk tiles so phase-B reads only wait on the chunk they
touch (finer deps let phase A overlap the previous iteration's tail). The
tail keeps only PSUM-touching work inline per q-block; the SBUF-only
rsqrt+scale+store runs once at iteration end, so the next iteration's
phase A isn't gated behind it by PSUM pool aliasing.

Fused tail: with t = (y-mu)*rs (LN0) and z = t + relu(t) (= rs*w where
w = max(2a, a), a = y-mu), LN1(z) = (w - mean(w)) * rsqrt(var(w) +
eps*(var(y)+eps)) -- LN0's rs never needs to be computed.

Knobs (env): ACTB=0 (ACT additionally takes score tile B every ACTB-th j),
ACOPY=split (phase-A copy engine: Kp/Vp evacs alternate ACT/DVE), AVLAG=2, UB (u ring, default from AVLAG).
"""

import os

import numpy as np

import concourse.bass as bass
import concourse.mybir as mybir
import concourse.tile as tile
from concourse import bacc
from concourse.bass_utils import run_bass_kernel_spmd

F32 = mybir.dt.float32
F32R = mybir.dt.float32r
BF16 = mybir.dt.bfloat16
AF = mybir.ActivationFunctionType
ALU = mybir.AluOpType

B, NQ, NK = 2, 4096, 4096
D = 256
H = 8
DH = 32
LN_EPS = 1e-5
NCORES = 8
QC = (B * NQ) // NCORES  # 1024 query rows per core
SCALE = 1.0 / np.sqrt(np.float32(DH))
NJ = NK // 128  # 32 k-tiles

_DVE_OPS = {}


def _register_dve_ops():
    """Runtime-register the custom DVE ops used by this kernel."""
    if _DVE_OPS:
        return _DVE_OPS
    import concourse.dve_ops as dve_ops
    from concourse.dve_spec import (
        C0, C1, C2, C3, Spec, Src0, _spill_c3_to_src1, lower,
    )
    from concourse.dve_uop import DveOpSpec

    def _mk(name, spec, rd1_en):
        for op in dve_ops.OPS:
            if op.name == name:
                return op
        row = dve_ops._CUSTOM_DVE_ROW_BASE + len(dve_ops.OPS)
        shas = {}
        for ver in ("v3", "v4"):
            tmp = DveOpSpec(name=name, opcode=row, uops=lower(spec, ver=ver),
                            rd1_en=rd1_en)
            shas[ver] = tmp.sha(ver)
        op = dve_ops.DveOp(name, spec, subdim=False, uops_sha=shas)
        dve_ops.OPS.append(op)
        dve_ops.CUSTOM_DVE_SPECS[op.name] = op.spec
        dve_ops._SUB_OPCODE_FOR_NAME[op.name] = row
        return op

    # cubic exp: out = ((c3*x + c2)*x + c1)*x + c0, c3 rides in1 ([P,1])
    def _exp3_ref(in0, in1, c0, c1, c2):
        c3 = in1[:, :1]
        x = in0.astype(np.float32)
        return ((c3 * x + c2) * x + c1) * x + c0

    exp3 = _mk(
        "EXP3_ANT",
        Spec(
            body=_spill_c3_to_src1(((C3 * Src0 + C2) * Src0 + C1) * Src0 + C0),
            reference=_exp3_ref,
        ),
        rd1_en=True,
    )
    _DVE_OPS["exp3"] = exp3
    return _DVE_OPS


def _fit_exp_cubic(scale, hi_raw):
    """Chebyshev-node cubic fit of e^(scale*x) for x in [-hi_raw, hi_raw]
    (raw, unscaled scores). Returns (c0, c1, c2, c3)."""
    t = np.cos(np.linspace(0, np.pi, 20001))
    xc = hi_raw * t
    yc = np.exp(np.float64(scale) * xc)
    c = np.polyfit(xc, yc, 3)
    return tuple(float(v) for v in c[::-1])


def _build_kernel(trivial_affine, repeat=1):
    """Build the SPMD Bass program. trivial_affine: all biases zero, all LN
    gammas one, betas zero (true for this problem's setup_inputs)."""
    ops = _register_dve_ops()
    exp3 = ops["exp3"]
    c0, c1, c2, c3 = _fit_exp_cubic(SCALE, 4.6)
    # ACT additionally takes score tile B every ACTB-th j (0 = never)
    ACTB = int(os.environ.get("ACTB", "0"))
    # phase-A PSUM->SBUF copy engine: act | dve
    ACOPY = os.environ.get("ACOPY", "split")
    # DVE additionally takes score tile A every DVEB-th j (0 = never)
    DVEB = int(os.environ.get("DVEB", "0"))
    AVLAG = int(os.environ.get("AVLAG", "2"))
    UB = int(os.environ.get("UB", str(2 * (int(os.environ.get("AVLAG", "2")) + 1))))

    nc = bacc.Bacc("TRN2", target_bir_lowering=False)

    # ---- dram i/o ----
    qT = nc.dram_tensor("qT", [D, QC], F32R, kind="ExternalInput")
    kT = nc.dram_tensor("kT", [D, NK], F32R, kind="ExternalInput")
    wqT = nc.dram_tensor("wqT", [D, D], F32R, kind="ExternalInput")
    wkT = nc.dram_tensor("wkT", [D, D], F32R, kind="ExternalInput")
    wvT = nc.dram_tensor("wvT", [D, D], F32R, kind="ExternalInput")
    woT = nc.dram_tensor("woT", [D, D], F32R, kind="ExternalInput")
    # vecsP[d, i]: per-partition-use vectors; col 0=bq, 1=bv
    vecsP = nc.dram_tensor("vecsP", [D, 2], F32, kind="ExternalInput")
    # vecsF[i, d]: free-dim-use vectors; row 0=bo 1=g0 2=beta0 3=g1 4=beta1
    vecsF = nc.dram_tensor("vecsF", [5, D], F32, kind="ExternalInput")
    out = nc.dram_tensor("out", [QC, D], F32, kind="ExternalOutput")

    with tile.TileContext(nc) as tc:
        with tc.tile_pool(name="sb", bufs=1) as sb:
            # ---- load inputs (q/k chunked so phase A streams behind) ----
            qt = [sb.tile([128, QC], F32R, tag=f"qt{i}", name=f"qt{i}") for i in range(2)]
            kt = [sb.tile([128, NK], F32R, tag=f"kt{i}", name=f"kt{i}") for i in range(2)]
            wqt = [sb.tile([128, D], F32R, tag=f"wqt{i}", name=f"wqt{i}") for i in range(2)]
            wkt = [sb.tile([128, D], F32R, tag=f"wkt{i}", name=f"wkt{i}") for i in range(2)]
            wvt = [sb.tile([128, D], F32R, tag=f"wvt{i}", name=f"wvt{i}") for i in range(2)]
            wot = [sb.tile([128, D], F32R, tag=f"wot{i}", name=f"wot{i}") for i in range(2)]
            ones32 = sb.tile([128, 32], BF16)
            c3t = sb.tile([128, 1], F32)
            epst = sb.tile([128, 1], F32)
            vp_ = [sb.tile([128, 2], F32, tag=f"vp_{i}", name=f"vp_{i}") for i in range(2)]
            vf_ = sb.tile([128, 5, D], F32) if not trivial_affine else None
            # per-tail-block persistent state (written each iter, read at end)
            wws = sb.tile([128, 8, D], F32)
            mvbs = sb.tile([128, 8, 2], F32)
            dds = sb.tile([128, 8], F32)
            for i in range(2):
                nc.sync.dma_start(out=wqt[i], in_=wqT[128 * i : 128 * i + 128, :])
                nc.sync.dma_start(out=wkt[i], in_=wkT[128 * i : 128 * i + 128, :])
                nc.sync.dma_start(out=wvt[i], in_=wvT[128 * i : 128 * i + 128, :])
                nc.sync.dma_start(out=wot[i], in_=woT[128 * i : 128 * i + 128, :])
                nc.sync.dma_start(out=qt[i], in_=qT[128 * i : 128 * i + 128, :])
                for ck in range(4):
                    nc.sync.dma_start(
                        out=kt[i][:, 1024 * ck : 1024 * ck + 1024],
                        in_=kT[128 * i : 128 * i + 128,
                              1024 * ck : 1024 * ck + 1024],
                    )
                nc.sync.dma_start(out=vp_[i], in_=vecsP[128 * i : 128 * i + 128, :])
            nc.vector.memset(ones32, 1.0)
            if vf_ is not None:
                nc.gpsimd.dma_start(
                    out=vf_, in_=vecsF[:, :].unsqueeze(0).broadcast_to([128, 5, D])
                )
            nc.vector.memset(c3t, c3)
            nc.vector.memset(epst, LN_EPS)
            # warm the ACT exp/ln table set while input DMAs stream
            warmt = sb.tile([128, 1], F32)
            nc.scalar.activation(out=warmt, in_=epst, func=AF.Exp)
            nc.scalar.activation(out=warmt, in_=warmt, func=AF.Ln)

            qpt = [sb.tile([128, QC], BF16, tag=f"qpt{g}", name=f"qpt{g}")
                   for g in range(2)]
            # kpt/vp split into per-chunk tiles so phase-B reads only wait
            # on the chunk they touch (finer deps -> phase A/B overlap)
            kptc = [
                [sb.tile([128, 1024], BF16, tag=f"kpt{g}_{c}", name=f"kpt{g}_{c}")
                 for c in range(4)]
                for g in range(2)
            ]
            vpc = [sb.tile([128, 4 * D], BF16, tag=f"vp{c}", name=f"vp{c}")
                   for c in range(8)]
            attnT = [
                sb.tile([128, QC], F32R, tag=f"attnT{g}", name=f"attnT{g}")
                for g in range(2)
            ]

            with (
                tc.tile_pool(name="upool", bufs=UB) as upool,
                tc.tile_pool(name="tails", bufs=3) as tails,
            ):

                def emit_iter():
                    # ONE psum pool for projections AND scores: phase A
                    # pipelines through the same 3-slot ring as phase B, so
                    # the first score matmuls are not barriered behind the
                    # last Vp evacuation by pool-stack bank aliasing
                    with (
                        tc.tile_pool(name="scp", bufs=3, space="PSUM") as scp,
                        tc.tile_pool(name="avp", bufs=1, space="PSUM") as avp,
                        tc.tile_pool(name="dnp", bufs=1, space="PSUM") as dnp,
                    ):
                        # ---- phase A: projections ----
                        # QpT: [dv-chunk g 128, q 1024]
                        for g in range(2):
                            qp_ps = scp.tile([128, QC], F32, tag="sc", name="qp")
                            for qb in range(2):
                                for dc in range(2):
                                    nc.tensor.matmul(
                                        qp_ps[:, 512 * qb : 512 * qb + 512],
                                        wqt[dc][:, 128 * g : 128 * g + 128],
                                        qt[dc][:, 512 * qb : 512 * qb + 512],
                                        start=(dc == 0),
                                        stop=(dc == 1),
                                    )
                            if trivial_affine:
                                if ACOPY == "dve":
                                    nc.vector.tensor_copy(qpt[g], qp_ps[:, :])
                                else:
                                    nc.scalar.activation(
                                        out=qpt[g], in_=qp_ps[:, :], func=AF.Copy
                                    )
                            else:
                                nc.vector.tensor_scalar(
                                    out=qpt[g], in0=qp_ps[:, :],
                                    scalar1=vp_[g][:, 0:1], scalar2=None,
                                    op0=ALU.add,
                                )
                        # KpT (K bias dropped: softmax-invariant per
                        # query); two kb chunks share a psum tile
                        for g in range(2):
                            for kb in range(0, 8, 2):
                                kp_ps = scp.tile([128, 1024], F32, tag="sc", name="kp")
                                for half in range(2):
                                    for dc in range(2):
                                        nc.tensor.matmul(
                                            kp_ps[:, 512 * half : 512 * half + 512],
                                            wkt[dc][:, 128 * g : 128 * g + 128],
                                            kt[dc][
                                                :,
                                                512 * (kb + half) : 512 * (kb + half) + 512,
                                            ],
                                            start=(dc == 0),
                                            stop=(dc == 1),
                                        )
                                if ACOPY == "dve" or (
                                    ACOPY == "split" and (kb // 2) % 2 == 1
                                ):
                                    nc.vector.tensor_copy(
                                        kptc[g][kb // 2], kp_ps[:, :]
                                    )
                                else:
                                    nc.scalar.activation(
                                        out=kptc[g][kb // 2],
                                        in_=kp_ps[:, :], func=AF.Copy,
                                    )
                        # Vp: [k-tile 128, dv 256] (V bias folded
                        # post-attention); four k-tiles share a psum tile
                        for kt_i in range(0, NJ, 4):
                            vps = scp.tile([128, 1024], F32, tag="sc", name="vps")
                            for half in range(4):
                                for dc in range(2):
                                    nc.tensor.matmul(
                                        vps[:, 256 * half : 256 * half + 256],
                                        kt[dc][
                                            :,
                                            128 * (kt_i + half) : 128 * (kt_i + half) + 128,
                                        ],
                                        wvt[dc][:, :],
                                        start=(dc == 0),
                                        stop=(dc == 1),
                                    )
                            if ACOPY == "dve" or (
                                ACOPY == "split" and (kt_i // 4) % 2 == 0
                            ):
                                nc.vector.tensor_copy(vpc[kt_i // 4], vps)
                            else:
                                nc.scalar.activation(
                                    out=vpc[kt_i // 4], in_=vps, func=AF.Copy,
                                )

                        # ---- phase B: attention ----
                        for qb in range(2):
                            for g in range(2):
                                av_ps = avp.tile([128, 512], F32, tag="av")
                                dn_ps = dnp.tile([128, 512], F32, tag="dn")
                                # av/dn run AVLAG j's behind the score/exp
                                # stream so PE never stalls on one exp
                                uq = []
                                for j in range(NJ + AVLAG):
                                    if j < NJ:
                                        st = [
                                            scp.tile([128, 1024], F32, tag="sc", name="sc")
                                            for _ in range(2)
                                        ]
                                        for hp in range(4):
                                            nc.tensor.matmul(
                                                st[hp // 2][
                                                    :, 512 * (hp % 2) : 512 * (hp % 2) + 512
                                                ],
                                                kptc[g][j // 8][
                                                    32 * hp : 32 * hp + 32,
                                                    128 * (j % 8) : 128 * (j % 8) + 128,
                                                ],
                                                qpt[g][
                                                    32 * hp : 32 * hp + 32,
                                                    512 * qb : 512 * qb + 512,
                                                ],
                                                start=True,
                                                stop=True,
                                                tile_position=(32 * hp, 0),
                                            )
                                        u = [
                                            upool.tile([128, 1024], BF16, tag="u", name="u")
                                            for _ in range(2)
                                        ]
                                        # tile A: ACT native exp; DVE
                                        # cubic every DVEB-th j
                                        if DVEB and j % DVEB == DVEB - 1:
                                            nc.vector._custom_dve(
                                                exp3, out=u[0], in0=st[0][:, :],
                                                in1=c3t, s0=c0, s1=c1, imm2=c2,
                                            )
                                        else:
                                            nc.scalar.activation(
                                                out=u[0], in_=st[0][:, :],
                                                func=AF.Exp, scale=float(SCALE),
                                            )
                                        # tile B: DVE cubic; ACT every ACTB-th j
                                        if ACTB and j % ACTB == ACTB - 1:
                                            nc.scalar.activation(
                                                out=u[1], in_=st[1][:, :],
                                                func=AF.Exp, scale=float(SCALE),
                                            )
                                        else:
                                            nc.vector._custom_dve(
                                                exp3, out=u[1], in0=st[1][:, :],
                                                in1=c3t, s0=c0, s1=c1, imm2=c2,
                                            )
                                        uq.append((u, j))
                                    if (j < NJ and len(uq) > AVLAG) or (
                                        j >= NJ and uq
                                    ):
                                        prev_u, jm = uq.pop(0)
                                        for hp in range(4):
                                            us = prev_u[hp // 2][
                                                :, 512 * (hp % 2) : 512 * (hp % 2) + 512
                                            ]
                                            nc.tensor.matmul(
                                                av_ps[32 * hp : 32 * hp + 32, :],
                                                vpc[jm // 4][
                                                    :,
                                                    256 * (jm % 4) + 128 * g + 32 * hp :
                                                    256 * (jm % 4) + 128 * g + 32 * hp + 32,
                                                ],
                                                us,
                                                start=(jm == 0),
                                                stop=(jm == NJ - 1),
                                                tile_position=(0, 32 * hp),
                                                skip_group_check=True,
                                            )
                                            nc.tensor.matmul(
                                                dn_ps[32 * hp : 32 * hp + 32, :],
                                                ones32[:, :],
                                                us,
                                                start=(jm == 0),
                                                stop=(jm == NJ - 1),
                                                tile_position=(0, 32 * hp),
                                                skip_group_check=True,
                                            )
                                # normalize: attnT = av * (1/den) [+ bv]
                                rden = tails.tile([128, 512], F32, tag="rden")
                                nc.vector.reciprocal_approx_fast(rden, dn_ps[:, :])
                                dst = attnT[g][:, 512 * qb : 512 * qb + 512]
                                nc.vector.tensor_mul(dst, av_ps[:, :], rden)
                                if not trivial_affine:
                                    nc.vector.tensor_scalar(
                                        out=dst, in0=dst, scalar1=vp_[g][:, 1:2],
                                        scalar2=None, op0=ALU.add,
                                    )

                            # ---- tail stats for this q-block (psum-touching
                            # work inline; SBUF-only rsqrt+scale deferred) ----
                            for t4 in range(4):
                                i8 = qb * 4 + t4
                                y_ps = scp.tile([128, 1024], F32, tag="sc", name="y")
                                yp = y_ps[:, 0:256]
                                q0 = 128 * i8
                                for g in range(2):
                                    nc.tensor.matmul(
                                        yp,
                                        attnT[g][:, q0 : q0 + 128],
                                        wot[g][:, :],
                                        start=(g == 0),
                                        stop=(g == 1),
                                    )
                                if trivial_affine:
                                    # fused tail: w = max(2a, a), a = y - mu;
                                    # out = (w - mu_w) * rsqrt(var_w +
                                    #        eps*(var_y + eps)); rsqrt batched
                                    # across all 8 blocks at iteration end
                                    st6 = tails.tile([128, 6], F32, tag="st6")
                                    mv = tails.tile([128, 2], F32, tag="mv")
                                    nc.vector.bn_stats(out=st6, in_=yp)
                                    nc.vector.bn_aggr(out=mv, in_=st6)
                                    # aa = yp - mu on ACT (bias add), -mu from
                                    # Pool: keeps DVE out of this step
                                    nmu = tails.tile([128, 1], F32, tag="nmu")
                                    nc.gpsimd.tensor_scalar(
                                        out=nmu, in0=mv[:, 0:1], scalar1=-1.0,
                                        scalar2=None, op0=ALU.mult,
                                    )
                                    aa = tails.tile([128, D], F32, tag="aa")
                                    nc.scalar.activation(
                                        out=aa, in_=yp, func=AF.Identity,
                                        bias=nmu[:, :],
                                    )
                                    # w = max(2a, a) = a + relu(a) on Pool
                                    zr = tails.tile([128, D], F32, tag="zr")
                                    nc.gpsimd.tensor_scalar(
                                        out=zr, in0=aa, scalar1=0.0,
                                        scalar2=None, op0=ALU.max,
                                    )
                                    nc.gpsimd.tensor_add(wws[:, i8, :], aa, zr)
                                    st6b = tails.tile([128, 6], F32, tag="st6b")
                                    nc.vector.bn_stats(out=st6b, in_=wws[:, i8, :])
                                    nc.vector.bn_aggr(out=mvbs[:, i8, :], in_=st6b)
                                    # d = var_w + eps*(var_y + eps)
                                    ddt = tails.tile([128, 1], F32, tag="ddt")
                                    nc.gpsimd.tensor_scalar(
                                        out=ddt, in0=mv[:, 1:2], scalar1=LN_EPS,
                                        scalar2=LN_EPS, op0=ALU.add, op1=ALU.mult,
                                    )
                                    nc.gpsimd.tensor_add(
                                        dds[:, i8 : i8 + 1], ddt, mvbs[:, i8, 1:2]
                                    )
                                else:
                                    nc.vector.tensor_add(yp, yp, vf_[:, 0, :])
                                    st6 = tails.tile([128, 6], F32, tag="st6")
                                    mv = tails.tile([128, 2], F32, tag="mv")
                                    rs = tails.tile([128, 1], F32, tag="rs")
                                    nc.vector.bn_stats(out=st6, in_=yp)
                                    nc.vector.bn_aggr(out=mv, in_=st6)
                                    nc.scalar.activation(
                                        out=rs, in_=mv[:, 1:2], func=AF.Ln,
                                        bias=epst[:, :],
                                    )
                                    nc.scalar.activation(
                                        out=rs, in_=rs, func=AF.Exp, scale=-0.5
                                    )
                                    h0 = tails.tile([128, D], F32, tag="h0")
                                    nc.vector.tensor_scalar(
                                        out=h0, in0=yp, scalar1=mv[:, 0:1],
                                        scalar2=rs, op0=ALU.subtract, op1=ALU.mult,
                                    )
                                    nc.vector.tensor_mul(h0, h0, vf_[:, 1, :])
                                    nc.vector.tensor_add(h0, h0, vf_[:, 2, :])
                                    zr = tails.tile([128, D], F32, tag="zr")
                                    nc.vector.tensor_scalar_max(zr, h0, 0.0)
                                    z = tails.tile([128, D], F32, tag="z")
                                    nc.vector.tensor_add(z, h0, zr)
                                    st6b = tails.tile([128, 6], F32, tag="st6b")
                                    mvb = tails.tile([128, 2], F32, tag="mvb")
                                    rsb = tails.tile([128, 1], F32, tag="rsb")
                                    nc.vector.bn_stats(out=st6b, in_=z)
                                    nc.vector.bn_aggr(out=mvb, in_=st6b)
                                    nc.scalar.activation(
                                        out=rsb, in_=mvb[:, 1:2], func=AF.Ln,
                                        bias=epst[:, :],
                                    )
                                    nc.scalar.activation(
                                        out=rsb, in_=rsb, func=AF.Exp, scale=-0.5
                                    )
                                    ot = tails.tile([128, D], F32, tag="ot")
                                    nc.vector.tensor_scalar(
                                        out=ot, in0=z, scalar1=mvb[:, 0:1],
                                        scalar2=rsb, op0=ALU.subtract, op1=ALU.mult,
                                    )
                                    nc.vector.tensor_mul(ot, ot, vf_[:, 3, :])
                                    nc.vector.tensor_add(ot, ot, vf_[:, 4, :])
                                    nc.sync.dma_start(
                                        out=out[q0 : q0 + 128, :], in_=ot
                                    )

                        if trivial_affine:
                            # batched rsqrt = exp(-0.5*ln(d)) for all 8 tail
                            # blocks in one Ln+Exp pair (SBUF-only epilogue)
                            rsa = tails.tile([128, 8], F32, tag="rsa")
                            nc.scalar.activation(out=rsa, in_=dds, func=AF.Ln)
                            nc.scalar.activation(
                                out=rsa, in_=rsa, func=AF.Exp, scale=-0.5
                            )
                            for i8 in range(8):
                                ot = tails.tile([128, D], F32, tag="ot")
                                nc.gpsimd.tensor_scalar(
                                    out=ot, in0=wws[:, i8, :],
                                    scalar1=mvbs[:, i8, 0:1],
                                    scalar2=rsa[:, i8 : i8 + 1],
                                    op0=ALU.subtract, op1=ALU.mult,
                                )
                                nc.sync.dma_start(
                                    out=out[128 * i8 : 128 * i8 + 128, :], in_=ot
                                )

                if repeat == 1:
                    emit_iter()
                elif os.environ.get("PYUNROLL") == "1":
                    # python-unrolled repeat: lets the no-exec TimelineSim
                    # measure steady-state (it cannot follow reg-mode loops)
                    for _ in range(repeat):
                        emit_iter()
                else:
                    with tc.For_i(0, repeat):
                        emit_iter()

    nc.compile()
    return nc


_KERNEL_CACHE = {}


def _get_kernel(trivial_affine, repeat=1):
    key = (
        bool(trivial_affine), int(repeat),
        os.environ.get("ACTB", "0"), os.environ.get("ACOPY", "split"),
        os.environ.get("UB", ""), os.environ.get("AVLAG", "2"),
        os.environ.get("DVEB", "0"),
    )
    if key not in _KERNEL_CACHE:
        _KERNEL_CACHE[key] = _build_kernel(key[0], key[1])
    return _KERNEL_CACHE[key]


def _prep(Q, K, Wq, bq, Wk, bk, Wv, bv, Wo, bo, g0, beta0, g1, beta1):
    """Shared input prep: returns (trivial_affine, in_maps)."""
    Q = np.asarray(Q, dtype=np.float32)
    K = np.asarray(K, dtype=np.float32)
    Wq = np.asarray(Wq, dtype=np.float32)
    Wk = np.asarray(Wk, dtype=np.float32)
    Wv = np.asarray(Wv, dtype=np.float32)
    Wo = np.asarray(Wo, dtype=np.float32)
    bq, bv, bo, g0, beta0, g1, beta1 = [
        np.asarray(v, dtype=np.float32)
        for v in (bq, bv, bo, g0, beta0, g1, beta1)
    ]

    trivial = bool(
        not bq.any() and not bv.any() and not bo.any()
        and not beta0.any() and not beta1.any()
        and np.all(g0 == 1.0) and np.all(g1 == 1.0)
    )

    wqTn = np.ascontiguousarray(Wq.T)
    wkTn = np.ascontiguousarray(Wk.T)
    wvTn = np.ascontiguousarray(Wv.T)
    woTn = np.ascontiguousarray(Wo.T)
    vecsP = np.stack([bq, bv], axis=1).astype(np.float32)  # [D, 2]
    vecsF = np.stack([bo, g0, beta0, g1, beta1], axis=0).astype(np.float32)

    kTb = [np.ascontiguousarray(K[b].T) for b in range(B)]
    in_maps = []
    for c in range(NCORES):
        b, qc = divmod(c, NCORES // B)
        in_maps.append(
            {
                "qT": np.ascontiguousarray(Q[b, QC * qc : QC * qc + QC, :].T),
                "kT": kTb[b],
                "wqT": wqTn,
                "wkT": wkTn,
                "wvT": wvTn,
                "woT": woTn,
                "vecsP": vecsP,
                "vecsF": vecsF,
            }
        )
    return trivial, in_maps


def _gather(res):
    outp = np.empty((B, NQ, D), dtype=np.float32)
    for c in range(NCORES):
        b, qc = divmod(c, NCORES // B)
        outp[b, QC * qc : QC * qc + QC, :] = res.results[c]["out"]
    return outp


def kernel(**inputs):
    trivial, in_maps = _prep(**inputs)
    nc = _get_kernel(trivial)
    res = run_bass_kernel_spmd(nc, in_maps, list(range(NCORES)))
    return _gather(res)
